# revision 29
# baseline (speedup 1.0000x reference)
"""Swin-style windowed attention on 8 TRN2 NeuronCores — v3.

Data-parallel over windows: core i handles windows [64i, 64i+64).

Phase 1 (qkv precompute, all 64 windows into big SBUF tensors):
  qk-proj: fp8e4 DoubleRow (contract 192 = 2x96), 2 windows per matmul
           -> QK[4][96, 64*196] bf16 (qA, qB, kA, kB per-half layout)
  v-proj:  bf16, x stationary, ones-augmented -> VA[98, 64*396] bf16
Phase 2 (attention, software pipeline over windows):
  S^T = k^T.T @ qT per (head, m-tile), head-pair PSUM tiles [98, 1024],
        tile_position rotation by head
  E = exp(S^T) per pair (ACT), P = E * ebm (DVE, ebm = exp(bias+mask))
  PV: O^T[(l,d+s), n] = vaug.T @ P per (head, m-tile) accumulated,
      heads packed at rows 33l of [99, 392] PSUM tile (s at rows 33l+32)
  U = evict O^T (1 copy);  s rows -> strided gpsimd DMA -> [98,12]
  r = 1/s (DVE), back-DMA -> [1,1176], partition_broadcast -> R[32,1176]
  zt = U * R per head (6 DVE muls) -> [96,196] per half
  y = zt.T @ wp -> [98, 384] PSUM -> bf16 out
Host: folds softmax scale into w_qkv, gathers rel-pos bias, builds EBM table,
adds b_proj at the end.
"""

import numpy as np
import ml_dtypes

import concourse.bass as bass
import concourse.mybir as mybir
import concourse.tile as tile
from concourse import bacc
from concourse import bass_utils

BF16 = mybir.dt.bfloat16
F32 = mybir.dt.float32
FP8 = mybir.dt.float8e4
DR = mybir.MatmulPerfMode.DoubleRow
NPBF16 = ml_dtypes.bfloat16
NPFP8 = ml_dtypes.float8_e4m3fn

B, N, C, H, HD, NG = 512, 196, 192, 6, 32, 64
NCORES = 8
WPC = B // NCORES
MT = 98

_CACHE = {}
DBGW = 1


def _build_nc():
    nc = bacc.Bacc("TRN2", target_bir_lowering=False, debug=False,
                   enable_asserts=False)

    xa_d = nc.dram_tensor("xa", [WPC // 2, 98, 784], BF16,
                          kind="ExternalInput").ap()
    ebm_d = nc.dram_tensor("ebm", [WPC, 98, 6 * 392], BF16,
                           kind="ExternalInput").ap()
    id_d = nc.dram_tensor("ident", [98, 98], BF16, kind="ExternalInput").ap()
    wqk_d = nc.dram_tensor("wqk", [96, 768], BF16, kind="ExternalInput").ap()
    wv_d = nc.dram_tensor("wv", [2, 98, 198], BF16, kind="ExternalInput").ap()
    wp_d = nc.dram_tensor("wp", [2, 96, 192], BF16, kind="ExternalInput").ap()
    out_d = nc.dram_tensor("out", [WPC, N, C], BF16, kind="ExternalOutput").ap()

    with tile.TileContext(nc) as tc:
        with tc.tile_pool(name="static", bufs=1) as static_pool:
            wqk_t = static_pool.tile([96, 768], BF16, tag="wqk")
            nc.sync.dma_start(wqk_t[:, :], wqk_d)
            id_t = static_pool.tile([98, 98], BF16, tag="ident")
            nc.sync.dma_start(id_t[:, :], id_d)
            wv_t = []
            for kt in range(2):
                t = static_pool.tile([98, 198], BF16, tag=f"wv{kt}")
                nc.sync.dma_start(t[:, :], wv_d[kt])
                wv_t.append(t)
            wp_t = []
            for kt in range(2):
                t = static_pool.tile([96, 192], BF16, tag=f"wp{kt}")
                nc.sync.dma_start(t[:, :], wp_d[kt])
                wp_t.append(t)

            # persistent qkv for all windows: qA, qB, kA, kB
            QK = [static_pool.tile([96, WPC * 196], BF16, tag=f"QK{c}",
                                   name=f"QK{c}")
                  for c in range(4)]
            VA = static_pool.tile([98, WPC * 396], BF16, tag="VA")

            # ---------------- phase 1: qkv precompute ----------------
            with (
                tc.tile_pool(name="xp", bufs=3) as xp_pool,
                tc.tile_pool(name="pj", bufs=8, space="PSUM") as pj_pool,
            ):
                for u in range(WPC // 2):
                    xp = xp_pool.tile([98, 784], BF16, tag="xp")
                    nc.sync.dma_start(xp[:, :], xa_d[u])
                    # moving: [96, kt 2, (w 2, n 196)]
                    mov = xp[0:96, :].rearrange("p (w k n) -> p w k n", w=2,
                                                k=2
                                                ).rearrange("p w k n -> p k w n")
                    for c in range(4):
                        ps = pj_pool.tile([128, 512], F32, tag="pj")
                        if True:  # bf16 qk-proj (fp8 exceeded err gate)
                            for kt in range(2):
                                nc.tensor.matmul(
                                    ps[0:96, 0:392].rearrange(
                                        "p (w x) -> p w x", w=2),
                                    wqk_t[:, 384 * kt + 96 * c:
                                          384 * kt + 96 * (c + 1)],
                                    mov[:, kt, :, :],
                                    start=(kt == 0), stop=(kt == 1),
                                )
                        else:
                            nc.tensor.matmul(
                                ps[0:96, 0:392].rearrange("p (w x) -> p w x", w=2),
                                wqk_t[:, :].rearrange("p (k f) -> p k f", k=2
                                                      )[:, :, 96 * c:96 * (c + 1)],
                                mov,
                                start=True, stop=True, perf_mode=DR,
                            )
                        dst = QK[c][:, u * 392: (u + 1) * 392]
                        if c % 2 == 0:
                            nc.vector.tensor_copy(dst, ps[0:96, 0:392])
                        else:
                            nc.scalar.copy(dst, ps[0:96, 0:392])
                    for wl in range(2):
                        ps = pj_pool.tile([128, 512], F32, tag="pj")
                        for mt in range(2):
                            for kt in range(2):
                                nc.tensor.matmul(
                                    ps[0:98, mt * 198: (mt + 1) * 198],
                                    xp[0:98, wl * 392 + kt * 196 + mt * 98:
                                       wl * 392 + kt * 196 + mt * 98 + 98],
                                    wv_t[kt][:, :],
                                    start=(kt == 0), stop=(kt == 1),
                                )
                        dst = VA[:, (2 * u + wl) * 396: (2 * u + wl + 1) * 396]
                        if wl == 0:
                            nc.vector.tensor_copy(dst, ps[0:98, 0:396])
                        else:
                            nc.scalar.copy(dst, ps[0:98, 0:396])

            # ---------------- phase 2: attention pipeline ----------------
            with (
                tc.tile_pool(name="ebm", bufs=3) as ebm_pool,
                tc.tile_pool(name="ee", bufs=2) as e_pool,
                tc.tile_pool(name="pp", bufs=2) as p_pool,
                tc.tile_pool(name="uu", bufs=3) as u_pool,
                tc.tile_pool(name="zt", bufs=3) as zt_pool,
                tc.tile_pool(name="rr", bufs=2) as r_pool,
                tc.tile_pool(name="ysb", bufs=3) as ysb_pool,
                tc.tile_pool(name="spsum", bufs=2, space="PSUM") as s_psum,
                tc.tile_pool(name="opsum", bufs=1, space="PSUM") as o_psum,
                tc.tile_pool(name="ypsum", bufs=2, space="PSUM") as y_psum,
            ):
                ebm_tiles = {}
                P_tiles = {}
                O_tiles = {}
                U_tiles = {}
                R_tiles = {}
                zt_tiles = {}

                def dma_ebm(w):
                    t = ebm_pool.tile([98, 6 * 392], BF16, tag="ebm")
                    nc.sync.dma_start(t[:, :], ebm_d[w])
                    ebm_tiles[w] = t

                E_tiles = {}

                def s_pair(w, pair, P, E):
                    """4 S matmuls for heads 2*pair, 2*pair+1 of window w.
                    exp lands in the per-window E tile; one merged DVE mul
                    produces P after the last pair."""
                    Sps = s_psum.tile([98, 1024], F32, tag="S")
                    for mt in range(2):
                        for hl in range(2):
                            h = 2 * pair + hl
                            hf, l = h // 3, h % 3
                            qt, kt_ = QK[hf], QK[2 + hf]
                            nc.tensor.matmul(
                                Sps[:, hl * 512 + mt * 196:
                                    hl * 512 + (mt + 1) * 196],
                                kt_[32 * l:32 * l + 32,
                                    w * 196 + mt * 98: w * 196 + mt * 98 + 98],
                                qt[32 * l:32 * l + 32, w * 196: (w + 1) * 196],
                                start=True, stop=True,
                            )
                    nc.scalar.activation(
                        E[:, pair * 784: (pair + 1) * 784].rearrange(
                            "p (h x) -> p h x", h=2),
                        Sps[:, :].rearrange("p (h x) -> p h x", h=2)[:, :, 0:392],
                        mybir.ActivationFunctionType.Exp)
                    if pair == 1:
                        nc.vector.tensor_mul(P[:, 0:1176], E[:, 0:1176],
                                             ebm_tiles[w][:, 0:1176])
                    elif pair == 2:
                        nc.vector.tensor_mul(P[:, 1176:2352],
                                             E[:, 1176:2352],
                                             ebm_tiles[w][:, 1176:2352])

                def pv(w):
                    """12 PV matmuls, one 1-bank tile per half: l0 rows 0-32
                    f0-195, l1 rows 64-96 f0-195, l2 rows 0-32 f196-391."""
                    Os = [o_psum.tile([97, 392], F32, tag=f"O{hf}",
                                      name=f"O{hf}") for hf in range(2)]
                    O_tiles[w] = Os
                    P = P_tiles[w]
                    for l in (0, 1, 2):
                        for hf in range(2):
                            h = 3 * hf + l
                            row = 64 if l == 1 else 0
                            fo = 196 if l == 2 else 0
                            for mt in range(2):
                                nc.tensor.matmul(
                                    Os[hf][row: row + 33, fo: fo + 196],
                                    VA[:, w * 396 + mt * 198 + 33 * h:
                                       w * 396 + mt * 198 + 33 * h + 33],
                                    P[:, h * 392 + mt * 196:
                                      h * 392 + (mt + 1) * 196],
                                    start=(mt == 0), stop=(mt == 1),
                                )

                def u_evict(w):
                    """One merged U tile: cols 0:784 = (hf, l0|l2, n) from
                    O rows 0-32; cols 784:1176 = (hf, n) l1 from rows 64-96.
                    Row 32 of U = all 6 softmax denominators."""
                    U = u_pool.tile([33, 1176], BF16, tag="U", name="U")
                    U_tiles[w] = U
                    Os = O_tiles[w]
                    nc.vector.tensor_copy(U[:, 0:392], Os[0][0:33, :])
                    nc.scalar.copy(U[:, 784:980], Os[0][64:97, 0:196])
                    nc.scalar.copy(U[:, 392:784], Os[1][0:33, :])
                    nc.scalar.copy(U[:, 980:1176], Os[1][64:97, 0:196])

                def bounce(w):
                    """s row of U -> [98,12] -> recip -> [1,1176] -> bcast.
                    Gather/scatter on sync HWDGE (cheap issue); only the
                    partition broadcast stays on gpsimd."""
                    U = U_tiles[w]
                    s_t = r_pool.tile([98, 12], BF16, tag="st", name="st")
                    nc.sync.dma_start(s_t[:, :], U[32:33, :])
                    r_t = r_pool.tile([98, 12], BF16, tag="rt", name="rt")
                    with nc.allow_low_precision(
                            reason="softmax recip; rel_err gate 2e-2"):
                        nc.vector.reciprocal(r_t[:, :], s_t[:, :])
                    r_row = r_pool.tile([1, 1176], BF16, tag="rrow", name="rrow")
                    nc.sync.dma_start(r_row[0:1, :], r_t[:, :])
                    R = r_pool.tile([32, 1176], BF16, tag="R", name="R")
                    nc.gpsimd.partition_broadcast(R[:, :], r_row[0:1, :])
                    R_tiles[w] = R

                def zt_stage(w):
                    """zt[32l+d, (hf,n)] = U[l-rows] * R, both hf halves per
                    DVE op via strided (hf, n) APs: l0/l2 at stride 392 in
                    U cols 0:784, l1 contiguous at 784:1176."""
                    U = U_tiles[w]
                    R = R_tiles[w]
                    zt = zt_pool.tile([96, 392], BF16, tag="zt", name="zt")
                    e2 = U[0:32, 0:784].rearrange("p (a b n) -> p a b n",
                                                  a=2, b=2)
                    r2 = R[:, 0:784].rearrange("p (a b n) -> p a b n",
                                               a=2, b=2)
                    for l in range(3):
                        dst = zt[32 * l: 32 * l + 32, :].rearrange(
                            "p (a n) -> p a n", a=2)
                        if l == 1:
                            nc.vector.tensor_mul(
                                dst,
                                U[0:32, 784:1176].rearrange(
                                    "p (a n) -> p a n", a=2),
                                R[:, 784:1176].rearrange(
                                    "p (a n) -> p a n", a=2))
                        else:
                            b = l // 2
                            nc.vector.tensor_mul(
                                dst, e2[:, :, b, :], r2[:, :, b, :])
                    zt_tiles[w] = zt

                def proj_stage(w):
                    Y = y_psum.tile([98, 384], F32, tag="Y", name="Y")
                    zt = zt_tiles[w]
                    for nt in range(2):
                        for kt in range(2):
                            nc.tensor.matmul(
                                Y[:, nt * 192: (nt + 1) * 192],
                                zt[:, kt * 196 + nt * 98:
                                   kt * 196 + nt * 98 + 98],
                                wp_t[kt][:, :],
                                start=(kt == 0), stop=(kt == 1),
                            )
                    ysb = ysb_pool.tile([98, 384], BF16, tag="ysb", name="ysb")
                    nc.scalar.copy(ysb[:, :], Y[:, :])
                    nc.sync.dma_start(
                        out_d[w].rearrange("(t p) c -> p t c", p=98),
                        ysb[:, :].rearrange("p (t c) -> p t c", t=2))

                dma_ebm(0)
                for i in range(WPC + 3):
                    if i + 1 < WPC:
                        dma_ebm(i + 1)
                    if i < WPC:
                        P = p_pool.tile([98, 6 * 392], BF16, tag="P", name="P")
                        P_tiles[i] = P
                        E = e_pool.tile([98, 6 * 392], BF16, tag="E", name="E")
                        E_tiles[i] = E
                        s_pair(i, 0, P, E)
                        if i >= 1:
                            pv(i - 1)
                        if i >= 3:
                            proj_stage(i - 3)
                        s_pair(i, 1, P, E)
                        if i >= 1:
                            u_evict(i - 1)
                            bounce(i - 1)
                        if i >= 2:
                            zt_stage(i - 2)
                        s_pair(i, 2, P, E)
                    else:
                        if i - 1 < WPC:
                            pv(i - 1)
                            u_evict(i - 1)
                            bounce(i - 1)
                        if 0 <= i - 2 < WPC:
                            zt_stage(i - 2)
                        if 0 <= i - 3 < WPC:
                            proj_stage(i - 3)
                    for dd, lag in ((ebm_tiles, 1), (P_tiles, 1), (E_tiles, 1),
                                    (O_tiles, 1), (U_tiles, 2), (R_tiles, 2),
                                    (zt_tiles, 3)):
                        dd.pop(i - lag - 1, None)
    nc.compile()
    return nc


def _host_precompute(x, w_qkv, w_proj, bias_table, mask, rel_index):
    scale = HD ** (-0.5)
    wq = np.array(w_qkv, np.float32).copy()
    wq[0:C] *= scale

    # xa[u, p, wl*392 + kt*196 + j] = x[2u+wl, j, kt*96 + p]; ones row 96/kt1
    xT = np.ascontiguousarray(np.transpose(np.asarray(x, np.float32), (0, 2, 1)))
    xa = np.zeros((B, 98, 392), np.float32)
    xa[:, 0:96, 0:196] = xT[:, 0:96]
    xa[:, 0:96, 196:392] = xT[:, 96:192]
    xa[:, 96, 196:392] = 1.0
    xa2 = np.ascontiguousarray(
        xa.reshape(B // 2, 2, 98, 392).transpose(0, 2, 1, 3).reshape(
            B // 2, 98, 784))

    # wqk8[p, kt*384 + f] = wq[f, kt*96+p], f in [0, 384)
    wqkT = wq[0:384].T  # [192, 384]
    wqk8 = np.concatenate([wqkT[0:96], wqkT[96:192]], axis=1)  # [96, 768]

    # wv[kt, p, 33h+d] = wq[384+32h+d, kt*96+p]; ones row kt1 p=96; row 97 pad
    wv = np.zeros((2, 98, 198), np.float32)
    wvT = wq[384:576].T
    for h in range(H):
        wv[0, 0:96, 33 * h: 33 * h + 32] = wvT[0:96, 32 * h: 32 * h + 32]
        wv[1, 0:96, 33 * h: 33 * h + 32] = wvT[96:192, 32 * h: 32 * h + 32]
        wv[1, 96, 33 * h + 32] = 1.0

    # wp[kt] = w_proj.T rows for heads 3kt..3kt+2
    wpT = np.asarray(w_proj, np.float32).T
    wp = np.stack([wpT[0:96], wpT[96:192]])

    # EBM[w, p, h*392 + mt*196 + n] = bias[n, m, h] + mask[w, n, m]  (log space;
    # accumulated into S on-device, exp(S+bm) then emits P directly)
    bias = np.asarray(bias_table, np.float32)[np.asarray(rel_index).reshape(-1)]
    bias = bias.reshape(N, N, H)
    biasT = np.transpose(bias, (2, 1, 0))
    maskT = np.transpose(np.asarray(mask, np.float32), (0, 2, 1))
    ebm = np.exp(biasT[None] + maskT[:, None])
    ebm = ebm.reshape(NG, H, 2, MT, N).transpose(0, 3, 1, 2, 4)
    ebm = np.ascontiguousarray(ebm.reshape(NG, MT, H * 392))

    return (xa2.astype(NPBF16), wqk8.astype(NPBF16),
            wv.astype(NPBF16), wp.astype(NPBF16), ebm.astype(NPBF16))


def kernel(x, w_qkv, w_proj, b_proj, bias_table, mask, rel_index):
    xa2, wqk8, wv, wp, ebm = _host_precompute(
        x, w_qkv, w_proj, bias_table, mask, rel_index)

    if "nc" not in _CACHE:
        _CACHE["nc"] = _build_nc()
    nc = _CACHE["nc"]

    upc = WPC // 2
    ident = np.eye(98, dtype=NPBF16)
    in_maps = []
    for c in range(NCORES):
        in_maps.append({
            "xa": np.ascontiguousarray(xa2[c * upc:(c + 1) * upc]),
            "ebm": ebm,
            "wqk": wqk8, "wv": wv, "wp": wp, "ident": ident,
        })

    res = bass_utils.run_bass_kernel_spmd(nc, in_maps, core_ids=list(range(NCORES)))
    out = np.concatenate([res.results[c]["out"] for c in range(NCORES)], axis=0)
    out = out.astype(np.float32) + np.asarray(b_proj, np.float32)[None, None, :]
    return out



# revision 32
# speedup vs baseline: 1.0187x; 1.0187x over previous
"""Swin-style windowed attention on 8 TRN2 NeuronCores — v3.

Data-parallel over windows: core i handles windows [64i, 64i+64).

Phase 1 (qkv precompute, all 64 windows into big SBUF tensors):
  qk-proj: fp8e4 DoubleRow (contract 192 = 2x96), 2 windows per matmul
           -> QK[4][96, 64*196] bf16 (qA, qB, kA, kB per-half layout)
  v-proj:  bf16, x stationary, ones-augmented -> VA[98, 64*396] bf16
Phase 2 (attention, software pipeline over windows):
  S^T = k^T.T @ qT per (head, m-tile), head-pair PSUM tiles [98, 1024],
        tile_position rotation by head
  E = exp(S^T) per pair (ACT), P = E * ebm (DVE, ebm = exp(bias+mask))
  PV: O^T[(l,d+s), n] = vaug.T @ P per (head, m-tile) accumulated,
      heads packed at rows 33l of [99, 392] PSUM tile (s at rows 33l+32)
  U = evict O^T (1 copy);  s rows -> strided gpsimd DMA -> [98,12]
  r = 1/s (DVE), back-DMA -> [1,1176], partition_broadcast -> R[32,1176]
  zt = U * R per head (6 DVE muls) -> [96,196] per half
  y = zt.T @ wp -> [98, 384] PSUM -> bf16 out
Host: folds softmax scale into w_qkv, gathers rel-pos bias, builds EBM table,
adds b_proj at the end.
"""

import numpy as np
import ml_dtypes

import concourse.bass as bass
import concourse.mybir as mybir
import concourse.tile as tile
from concourse import bacc
from concourse import bass_utils

BF16 = mybir.dt.bfloat16
F32 = mybir.dt.float32
FP8 = mybir.dt.float8e4
DR = mybir.MatmulPerfMode.DoubleRow
NPBF16 = ml_dtypes.bfloat16
NPFP8 = ml_dtypes.float8_e4m3fn

B, N, C, H, HD, NG = 512, 196, 192, 6, 32, 64
NCORES = 8
WPC = B // NCORES
MT = 98

_CACHE = {}
DBGW = 1


def _build_nc():
    nc = bacc.Bacc("TRN2", target_bir_lowering=False, debug=False,
                   enable_asserts=False)

    xa_d = nc.dram_tensor("xa", [WPC // 2, 98, 784], BF16,
                          kind="ExternalInput").ap()
    ebm_d = nc.dram_tensor("ebm", [WPC, 98, 6 * 392], BF16,
                           kind="ExternalInput").ap()
    id_d = nc.dram_tensor("ident", [98, 98], BF16, kind="ExternalInput").ap()
    wqk_d = nc.dram_tensor("wqk", [96, 768], BF16, kind="ExternalInput").ap()
    wv_d = nc.dram_tensor("wv", [2, 98, 198], BF16, kind="ExternalInput").ap()
    wp_d = nc.dram_tensor("wp", [2, 96, 192], BF16, kind="ExternalInput").ap()
    out_d = nc.dram_tensor("out", [WPC, N, C], BF16, kind="ExternalOutput").ap()

    with tile.TileContext(nc) as tc:
        with tc.tile_pool(name="static", bufs=1) as static_pool:
            wqk_t = static_pool.tile([96, 768], BF16, tag="wqk")
            nc.sync.dma_start(wqk_t[:, :], wqk_d)
            id_t = static_pool.tile([98, 98], BF16, tag="ident")
            nc.sync.dma_start(id_t[:, :], id_d)
            wv_t = []
            for kt in range(2):
                t = static_pool.tile([98, 198], BF16, tag=f"wv{kt}")
                nc.sync.dma_start(t[:, :], wv_d[kt])
                wv_t.append(t)
            wp_t = []
            for kt in range(2):
                t = static_pool.tile([96, 192], BF16, tag=f"wp{kt}")
                nc.sync.dma_start(t[:, :], wp_d[kt])
                wp_t.append(t)

            # persistent qkv for all windows: qA, qB, kA, kB
            QK = [static_pool.tile([96, WPC * 196], BF16, tag=f"QK{c}",
                                   name=f"QK{c}")
                  for c in range(4)]
            VA = static_pool.tile([98, WPC * 396], BF16, tag="VA")

            # ---------------- phase 1: qkv precompute ----------------
            with (
                tc.tile_pool(name="xp", bufs=3) as xp_pool,
                tc.tile_pool(name="pj", bufs=8, space="PSUM") as pj_pool,
            ):
                for u in range(WPC // 2):
                    xp = xp_pool.tile([98, 784], BF16, tag="xp")
                    nc.sync.dma_start(xp[:, :], xa_d[u])
                    # moving: [96, kt 2, (w 2, n 196)]
                    mov = xp[0:96, :].rearrange("p (w k n) -> p w k n", w=2,
                                                k=2
                                                ).rearrange("p w k n -> p k w n")
                    for c in range(4):
                        ps = pj_pool.tile([128, 512], F32, tag="pj")
                        if True:  # bf16 qk-proj (fp8 exceeded err gate)
                            for kt in range(2):
                                nc.tensor.matmul(
                                    ps[0:96, 0:392].rearrange(
                                        "p (w x) -> p w x", w=2),
                                    wqk_t[:, 384 * kt + 96 * c:
                                          384 * kt + 96 * (c + 1)],
                                    mov[:, kt, :, :],
                                    start=(kt == 0), stop=(kt == 1),
                                )
                        else:
                            nc.tensor.matmul(
                                ps[0:96, 0:392].rearrange("p (w x) -> p w x", w=2),
                                wqk_t[:, :].rearrange("p (k f) -> p k f", k=2
                                                      )[:, :, 96 * c:96 * (c + 1)],
                                mov,
                                start=True, stop=True, perf_mode=DR,
                            )
                        dst = QK[c][:, u * 392: (u + 1) * 392]
                        if c % 2 == 0:
                            nc.vector.tensor_copy(dst, ps[0:96, 0:392])
                        else:
                            nc.scalar.copy(dst, ps[0:96, 0:392])
                    for wl in range(2):
                        ps = pj_pool.tile([128, 512], F32, tag="pj")
                        for mt in range(2):
                            for kt in range(2):
                                nc.tensor.matmul(
                                    ps[0:98, mt * 198: (mt + 1) * 198],
                                    xp[0:98, wl * 392 + kt * 196 + mt * 98:
                                       wl * 392 + kt * 196 + mt * 98 + 98],
                                    wv_t[kt][:, :],
                                    start=(kt == 0), stop=(kt == 1),
                                )
                        dst = VA[:, (2 * u + wl) * 396: (2 * u + wl + 1) * 396]
                        if wl == 0:
                            nc.vector.tensor_copy(dst, ps[0:98, 0:396])
                        else:
                            nc.scalar.copy(dst, ps[0:98, 0:396])

            # ---------------- phase 2: attention pipeline ----------------
            with (
                tc.tile_pool(name="ebm", bufs=3) as ebm_pool,
                tc.tile_pool(name="ee", bufs=2) as e_pool,
                tc.tile_pool(name="pp", bufs=2) as p_pool,
                tc.tile_pool(name="uu", bufs=3) as u_pool,
                tc.tile_pool(name="zt", bufs=3) as zt_pool,
                tc.tile_pool(name="rr", bufs=2) as r_pool,
                tc.tile_pool(name="ysb", bufs=3) as ysb_pool,
                tc.tile_pool(name="spsum", bufs=2, space="PSUM") as s_psum,
                tc.tile_pool(name="opsum", bufs=1, space="PSUM") as o_psum,
                tc.tile_pool(name="ypsum", bufs=2, space="PSUM") as y_psum,
            ):
                ebm_tiles = {}
                P_tiles = {}
                O_tiles = {}
                U_tiles = {}
                R_tiles = {}
                zt_tiles = {}

                def dma_ebm(w):
                    t = ebm_pool.tile([98, 6 * 392], BF16, tag="ebm")
                    nc.sync.dma_start(t[:, :], ebm_d[w])
                    ebm_tiles[w] = t

                E_tiles = {}

                def s_pair(w, pair, P, E):
                    """4 S matmuls for heads 2*pair, 2*pair+1 of window w.
                    exp lands in the per-window E tile; one merged DVE mul
                    produces P after the last pair."""
                    Sps = s_psum.tile([98, 1024], F32, tag="S")
                    for mt in range(2):
                        for hl in range(2):
                            h = 2 * pair + hl
                            hf, l = h // 3, h % 3
                            qt, kt_ = QK[hf], QK[2 + hf]
                            nc.tensor.matmul(
                                Sps[:, hl * 512 + mt * 196:
                                    hl * 512 + (mt + 1) * 196],
                                kt_[32 * l:32 * l + 32,
                                    w * 196 + mt * 98: w * 196 + mt * 98 + 98],
                                qt[32 * l:32 * l + 32, w * 196: (w + 1) * 196],
                                start=True, stop=True,
                            )
                    nc.scalar.activation(
                        E[:, pair * 784: (pair + 1) * 784].rearrange(
                            "p (h x) -> p h x", h=2),
                        Sps[:, :].rearrange("p (h x) -> p h x", h=2)[:, :, 0:392],
                        mybir.ActivationFunctionType.Exp)
                    if pair == 1:
                        nc.vector.tensor_mul(P[:, 0:1176], E[:, 0:1176],
                                             ebm_tiles[w][:, 0:1176])
                    elif pair == 2:
                        nc.vector.tensor_mul(P[:, 1176:2352],
                                             E[:, 1176:2352],
                                             ebm_tiles[w][:, 1176:2352])

                def pv(w):
                    """12 PV matmuls, one 1-bank tile per half: l0 rows 0-32
                    f0-195, l1 rows 64-96 f0-195, l2 rows 0-32 f196-391."""
                    Os = [o_psum.tile([97, 392], F32, tag=f"O{hf}",
                                      name=f"O{hf}") for hf in range(2)]
                    O_tiles[w] = Os
                    P = P_tiles[w]
                    for l in (0, 1, 2):
                        for hf in range(2):
                            h = 3 * hf + l
                            row = 64 if l == 1 else 0
                            fo = 196 if l == 2 else 0
                            for mt in range(2):
                                nc.tensor.matmul(
                                    Os[hf][row: row + 33, fo: fo + 196],
                                    VA[:, w * 396 + mt * 198 + 33 * h:
                                       w * 396 + mt * 198 + 33 * h + 33],
                                    P[:, h * 392 + mt * 196:
                                      h * 392 + (mt + 1) * 196],
                                    start=(mt == 0), stop=(mt == 1),
                                )

                def u_evict(w):
                    """One merged U tile: cols 0:784 = (hf, l0|l2, n) from
                    O rows 0-32; cols 784:1176 = (hf, n) l1 from rows 64-96.
                    Row 32 of U = all 6 softmax denominators."""
                    U = u_pool.tile([33, 1176], BF16, tag="U", name="U")
                    U_tiles[w] = U
                    Os = O_tiles[w]
                    nc.vector.tensor_copy(U[:, 0:392], Os[0][0:33, :])
                    nc.scalar.copy(U[:, 784:980], Os[0][64:97, 0:196])
                    nc.scalar.copy(U[:, 392:784], Os[1][0:33, :])
                    nc.scalar.copy(U[:, 980:1176], Os[1][64:97, 0:196])

                def bounce(w):
                    """s row of U -> [98,12] -> recip -> [1,1176] -> bcast.
                    Gather/scatter on sync HWDGE (cheap issue); only the
                    partition broadcast stays on gpsimd."""
                    U = U_tiles[w]
                    s_t = r_pool.tile([98, 12], BF16, tag="st", name="st")
                    nc.sync.dma_start(s_t[:, :], U[32:33, :])
                    r_t = r_pool.tile([98, 12], BF16, tag="rt", name="rt")
                    with nc.allow_low_precision(
                            reason="softmax recip; rel_err gate 2e-2"):
                        nc.vector.reciprocal(r_t[:, :], s_t[:, :])
                    r_row = r_pool.tile([1, 1176], BF16, tag="rrow", name="rrow")
                    nc.sync.dma_start(r_row[0:1, :], r_t[:, :])
                    R = r_pool.tile([32, 1176], BF16, tag="R", name="R")
                    nc.gpsimd.partition_broadcast(R[:, :], r_row[0:1, :])
                    R_tiles[w] = R

                def zt_stage(w):
                    """zt[32l+d, (hf,n)] = U[l-rows] * R, both hf halves per
                    DVE op via strided (hf, n) APs: l0/l2 at stride 392 in
                    U cols 0:784, l1 contiguous at 784:1176."""
                    U = U_tiles[w]
                    R = R_tiles[w]
                    zt = zt_pool.tile([96, 392], BF16, tag="zt", name="zt")
                    e2 = U[0:32, 0:784].rearrange("p (a b n) -> p a b n",
                                                  a=2, b=2)
                    r2 = R[:, 0:784].rearrange("p (a b n) -> p a b n",
                                               a=2, b=2)
                    for l in range(3):
                        dst = zt[32 * l: 32 * l + 32, :].rearrange(
                            "p (a n) -> p a n", a=2)
                        if l == 1:
                            nc.vector.tensor_mul(
                                dst,
                                U[0:32, 784:1176].rearrange(
                                    "p (a n) -> p a n", a=2),
                                R[:, 784:1176].rearrange(
                                    "p (a n) -> p a n", a=2))
                        else:
                            b = l // 2
                            nc.vector.tensor_mul(
                                dst, e2[:, :, b, :], r2[:, :, b, :])
                    zt_tiles[w] = zt

                def proj_stage(w):
                    Y = y_psum.tile([98, 384], F32, tag="Y", name="Y")
                    zt = zt_tiles[w]
                    for nt in range(2):
                        for kt in range(2):
                            nc.tensor.matmul(
                                Y[:, nt * 192: (nt + 1) * 192],
                                zt[:, kt * 196 + nt * 98:
                                   kt * 196 + nt * 98 + 98],
                                wp_t[kt][:, :],
                                start=(kt == 0), stop=(kt == 1),
                            )
                    ysb = ysb_pool.tile([98, 384], BF16, tag="ysb", name="ysb")
                    nc.vector.tensor_copy(ysb[:, :], Y[:, :])
                    nc.sync.dma_start(
                        out_d[w].rearrange("(t p) c -> p t c", p=98),
                        ysb[:, :].rearrange("p (t c) -> p t c", t=2))

                dma_ebm(0)
                for i in range(WPC + 3):
                    if i < WPC:
                        P = p_pool.tile([98, 6 * 392], BF16, tag="P", name="P")
                        P_tiles[i] = P
                        E = e_pool.tile([98, 6 * 392], BF16, tag="E", name="E")
                        E_tiles[i] = E
                        s_pair(i, 0, P, E)
                        if i >= 1:
                            pv(i - 1)
                        if i >= 3:
                            proj_stage(i - 3)
                        s_pair(i, 1, P, E)
                        if i >= 1:
                            u_evict(i - 1)
                            bounce(i - 1)
                        if i >= 2:
                            zt_stage(i - 2)
                        if i + 1 < WPC:
                            dma_ebm(i + 1)
                        s_pair(i, 2, P, E)
                    else:
                        if i - 1 < WPC:
                            pv(i - 1)
                            u_evict(i - 1)
                            bounce(i - 1)
                        if 0 <= i - 2 < WPC:
                            zt_stage(i - 2)
                        if 0 <= i - 3 < WPC:
                            proj_stage(i - 3)
                    for dd, lag in ((ebm_tiles, 1), (P_tiles, 1), (E_tiles, 1),
                                    (O_tiles, 1), (U_tiles, 2), (R_tiles, 2),
                                    (zt_tiles, 3)):
                        dd.pop(i - lag - 1, None)
    nc.compile()
    return nc


def _host_precompute(x, w_qkv, w_proj, bias_table, mask, rel_index):
    scale = HD ** (-0.5)
    wq = np.array(w_qkv, np.float32).copy()
    wq[0:C] *= scale

    # xa[u, p, wl*392 + kt*196 + j] = x[2u+wl, j, kt*96 + p]; ones row 96/kt1
    xT = np.ascontiguousarray(np.transpose(np.asarray(x, np.float32), (0, 2, 1)))
    xa = np.zeros((B, 98, 392), np.float32)
    xa[:, 0:96, 0:196] = xT[:, 0:96]
    xa[:, 0:96, 196:392] = xT[:, 96:192]
    xa[:, 96, 196:392] = 1.0
    xa2 = np.ascontiguousarray(
        xa.reshape(B // 2, 2, 98, 392).transpose(0, 2, 1, 3).reshape(
            B // 2, 98, 784))

    # wqk8[p, kt*384 + f] = wq[f, kt*96+p], f in [0, 384)
    wqkT = wq[0:384].T  # [192, 384]
    wqk8 = np.concatenate([wqkT[0:96], wqkT[96:192]], axis=1)  # [96, 768]

    # wv[kt, p, 33h+d] = wq[384+32h+d, kt*96+p]; ones row kt1 p=96; row 97 pad
    wv = np.zeros((2, 98, 198), np.float32)
    wvT = wq[384:576].T
    for h in range(H):
        wv[0, 0:96, 33 * h: 33 * h + 32] = wvT[0:96, 32 * h: 32 * h + 32]
        wv[1, 0:96, 33 * h: 33 * h + 32] = wvT[96:192, 32 * h: 32 * h + 32]
        wv[1, 96, 33 * h + 32] = 1.0

    # wp[kt] = w_proj.T rows for heads 3kt..3kt+2
    wpT = np.asarray(w_proj, np.float32).T
    wp = np.stack([wpT[0:96], wpT[96:192]])

    # EBM[w, p, h*392 + mt*196 + n] = bias[n, m, h] + mask[w, n, m]  (log space;
    # accumulated into S on-device, exp(S+bm) then emits P directly)
    bias = np.asarray(bias_table, np.float32)[np.asarray(rel_index).reshape(-1)]
    bias = bias.reshape(N, N, H)
    biasT = np.transpose(bias, (2, 1, 0))
    maskT = np.transpose(np.asarray(mask, np.float32), (0, 2, 1))
    ebm = np.exp(biasT[None] + maskT[:, None])
    ebm = ebm.reshape(NG, H, 2, MT, N).transpose(0, 3, 1, 2, 4)
    ebm = np.ascontiguousarray(ebm.reshape(NG, MT, H * 392))

    return (xa2.astype(NPBF16), wqk8.astype(NPBF16),
            wv.astype(NPBF16), wp.astype(NPBF16), ebm.astype(NPBF16))


def kernel(x, w_qkv, w_proj, b_proj, bias_table, mask, rel_index):
    xa2, wqk8, wv, wp, ebm = _host_precompute(
        x, w_qkv, w_proj, bias_table, mask, rel_index)

    if "nc" not in _CACHE:
        _CACHE["nc"] = _build_nc()
    nc = _CACHE["nc"]

    upc = WPC // 2
    ident = np.eye(98, dtype=NPBF16)
    in_maps = []
    for c in range(NCORES):
        in_maps.append({
            "xa": np.ascontiguousarray(xa2[c * upc:(c + 1) * upc]),
            "ebm": ebm,
            "wqk": wqk8, "wv": wv, "wp": wp, "ident": ident,
        })

    res = bass_utils.run_bass_kernel_spmd(nc, in_maps, core_ids=list(range(NCORES)))
    out = np.concatenate([res.results[c]["out"] for c in range(NCORES)], axis=0)
    out = out.astype(np.float32) + np.asarray(b_proj, np.float32)[None, None, :]
    return out

